# revision 1
# baseline (speedup 1.0000x reference)
"""Trainium2 Bass kernel for the nms_detection competition problem.

Computes, for inputs plateau [2,256,256,32], phenotypes [2,128,32],
positions [2,128,2], alive [2,128,1]:

    masks   = relu(normalize(plateau_flat) @ normalize(phenotypes)^T)   [B,N,P]
    I       = (masks>.5)^T (masks>.5) over N  -> iou -> disputes -> alive'
    out     = masks * alive'^T

Sharding: 8 cores = 2 batches x 4 pixel shards (16384 pixels each).

Per-core pipeline (32 chunks of 512 pixels):
  - host pre-transposes the plateau slice into qT[32j+q, 128c+p] (bf16)
    so the PE needs no on-device transposes; pixel n = 512c + 4p + j.
  - per-pixel sum-of-squares via a tiny E4 matmul (partition reduction),
    sqrt + reciprocal -> invn columns.
  - mask matmul in bf16 against a block-diagonal knT (K=128, N=512),
    PSUM evicted with fused per-partition scale (invn) + relu straight
    to bf16 SBUF tiles, DMA'd out (bf16 halves the write traffic; the
    host upcasts to f32).
  - binary masks (mask > 0.5, bf16) feed the I-gram accumulation
    matmuls (4 per chunk, 2 alternating PSUM banks).
  - [128,128] I partials are AllGather'd within each 4-core batch
    group; the compete logic runs redundantly per core.
  - masks are written optimistically (no alive filter); the host
    applies a device fix-up kernel only if some agent was killed.
"""
import os
import numpy as np
import ml_dtypes

import concourse.bass as bass
import concourse.tile as tile
from concourse import mybir
from concourse import bass_utils
from concourse.masks import make_identity
from contextlib import ExitStack

F32 = mybir.dt.float32
I32 = mybir.dt.int32
BF16 = mybir.dt.bfloat16

B, H, W, Q, P = 2, 256, 256, 32, 128
N = H * W                 # 65536 pixels per batch
NSHARD = 4                # pixel shards per batch
NCORE_PIX = N // NSHARD   # 16384 pixels per core
NCHUNK = 32               # chunks per core
CHUNK_PIX = NCORE_PIX // NCHUNK  # 512 pixels per chunk
N_CORES = 8

MASK_THRESH = 0.5
COMPETE_THRESH = 0.2
EPS = 1e-6
TWO23 = 8388608.0  # 2^23, for exact floor()

AluOp = mybir.AluOpType
ActFn = mybir.ActivationFunctionType


# ---------------------------------------------------------------------------
# Environment patches (walrus build here rejects >1 sync wait per instruction
# on the NO_STRUCT/S3_LW paths)
# ---------------------------------------------------------------------------
def _install_patches():
    if getattr(tile.TileContext, "_nms_drain_patched", False):
        return

    def _split_multiwaits(nc):
        """walrus here accepts at most one sync wait per instruction; move
        extra waits onto preceding same-engine NoOps."""
        ctr = [0]
        for bb in nc.main_func.blocks:
            insts = list(bb.instructions)
            if not any(i.sync_info is not None and len(i.sync_info.on_wait) > 1
                       for i in insts):
                continue
            new = []
            for inst in insts:
                si = inst.sync_info
                if si is not None and len(si.on_wait) > 1:
                    waits = list(si.on_wait)
                    for w in waits[:-1]:
                        ctr[0] += 1
                        nop = mybir.InstNoOp(
                            name=f"{inst.name}_wsplit{ctr[0]}",
                            engine=inst.engine,
                            bass_nofuse=True,
                            sync_info=mybir.SyncInfo(on_wait=[w], on_update=[]),
                        )
                        nc.register_instruction(nop, overwrite=True)
                        new.append(nop)
                    inst.sync_info = mybir.SyncInfo(
                        on_wait=[waits[-1]], on_update=list(si.on_update))
                new.append(inst)
            bb.instructions = new

    def _patched(self, tick_clock, wait_clock):
        from concourse.tile import ScopedClock
        drain_inst = self.nc.sync.drain()
        wait_clock.add_sem_waits(
            drain_inst.ins, ScopedClock({None: tick_clock.global_clock})
        )
        self.nc.all_engine_barrier()
        assert self.sems is not None
        popped = self.nc._tile_sem_poison_stack.pop()
        assert popped is self._sem_poison
        self.nc.clear_and_free_semaphores(list(self.sems.allocated().values()))
        self.nc.all_engine_barrier()
        _split_multiwaits(self.nc)

    tile.TileContext._drain_and_barrier = _patched
    tile.TileContext._nms_drain_patched = True

    # artifact upload would try to reach a share; keep everything local
    bass_utils.upload_artifacts = lambda tmpdir: tmpdir


_install_patches()


def _bcast_free(ap, reps):
    """AP view repeating each element of `ap` `reps` times along a new
    innermost free dim (step 0)."""
    return bass.AP(
        tensor=ap.tensor,
        offset=ap.offset,
        ap=list(ap.ap) + [[0, reps]],
    )


def build_kernel():
    nc = bass.Bass("TRN2", target_bir_lowering=False, debug=False,
                   enable_asserts=False, num_devices=N_CORES)

    # qT[32j+q, 128c+p] = plateau[b, base + 512c + 4p + j, q]  (host-built)
    qT_in = nc.dram_tensor("qT", [128, NCHUNK * 128], BF16,
                           kind="ExternalInput").ap()
    plateau = nc.dram_tensor("plateau", [N, Q], F32, kind="ExternalInput").ap()
    phen = nc.dram_tensor("phenotypes", [P, Q], F32, kind="ExternalInput").ap()
    pos = nc.dram_tensor("positions", [P, 2], F32, kind="ExternalInput").ap()
    alive = nc.dram_tensor("alive", [P, 1], F32, kind="ExternalInput").ap()
    out = nc.dram_tensor("out", [NCORE_PIX, P], BF16, kind="ExternalOutput").ap()
    alive_out = nc.dram_tensor("alive_out", [P, 1], F32, kind="ExternalOutput").ap()

    # pixel n = 512c + 4p + j  <->  (chunk c, partition p, subrow j)
    # pair two chunks per DMA: per partition 2 contiguous 1KiB bf16 blocks
    def out_pair_view(cc):
        return bass.AP(tensor=out.tensor, offset=cc * 2 * CHUNK_PIX * P,
                       ap=[[4 * P, 128], [CHUNK_PIX * P, 2], [1, 4 * P]])

    with tile.TileContext(nc) as tc, ExitStack() as ctx:
        singles = ctx.enter_context(tc.tile_pool(name="singles", bufs=1))
        mpool = ctx.enter_context(tc.tile_pool(name="mpool", bufs=4))
        mbpool = ctx.enter_context(tc.tile_pool(name="mbpool", bufs=4))
        qpool = ctx.enter_context(tc.tile_pool(name="qpool", bufs=3))
        small = ctx.enter_context(tc.tile_pool(name="small", bufs=3))
        ps = ctx.enter_context(tc.tile_pool(name="ps", bufs=1, space="PSUM"))
        psn = ctx.enter_context(tc.tile_pool(name="psn", bufs=1, space="PSUM"))
        psb = ctx.enter_context(tc.tile_pool(name="psb", bufs=1, space="PSUM"))
        psmm = ctx.enter_context(tc.tile_pool(name="psmm", bufs=3, space="PSUM"))
        psacc = ctx.enter_context(tc.tile_pool(name="psacc", bufs=1, space="PSUM"))
        dram = ctx.enter_context(tc.tile_pool(name="dram", bufs=1, space="DRAM"))
        p2 = ctx.enter_context(tc.tile_pool(name="p2", bufs=1))

        v, sc, gp, te = nc.vector, nc.scalar, nc.gpsimd, nc.tensor

        # ------------------------------------------------------------------
        # inputs first: stream qT in, tiny tensors; then prep
        # ------------------------------------------------------------------
        qTall = singles.tile([128, NCHUNK * 128], BF16)
        for g in range(4):
            lo, hi = g * 8 * 128, (g + 1) * 8 * 128
            nc.sync.dma_start(out=qTall[:, lo:hi], in_=qT_in[:, lo:hi])
        ph = singles.tile([P, Q], F32)
        nc.sync.dma_start(out=ph[:], in_=phen)
        alive_in = singles.tile([P, 1], F32)
        nc.sync.dma_start(out=alive_in[:], in_=alive)
        posb = singles.tile([P, 2], F32)
        nc.sync.dma_start(out=posb[:], in_=pos)

        # primer collective (first gpsimd instruction): absorbs the global
        # device barrier + CC ring setup so the real AllGather at the end
        # starts promptly; nothing consumes dcout.
        djunk = singles.tile([128, 4], F32)
        v.memset(djunk[:], 0.0)
        dcin = dram.tile([128, 4], F32)
        dcout = dram.tile([4 * 128, 4], F32)
        nc.sync.dma_start(out=dcin[:], in_=djunk[:])
        gp.collective_compute(
            "AllGather", AluOp.bypass,
            replica_groups=[[0, 1, 2, 3], [4, 5, 6, 7]],
            ins=[dcin[:].opt()], outs=[dcout[:].opt()],
        )

        # scalar activation-table preload (overlaps input DMA)
        junk1 = singles.tile([1, 4], F32)
        v.memset(junk1[:], 1.0)
        junk1b = singles.tile([1, 4], F32)
        sc.sqrt(out=junk1b[:], in_=junk1[:])

        # ------------------------------------------------------------------
        # prep: identity, phenotypes -> kn, block-diagonal KD (bf16), E4
        # ------------------------------------------------------------------
        ident = singles.tile([128, 128], F32)
        make_identity(nc, ident[:])

        sqk = small.tile([P, Q], F32)
        v.tensor_tensor(out=sqk[:], in0=ph[:], in1=ph[:], op=AluOp.mult)
        nk = small.tile([P, 1], F32)
        v.reduce_sum(out=nk[:], in_=sqk[:], axis=mybir.AxisListType.X)
        sc.sqrt(out=nk[:], in_=nk[:])
        v.tensor_scalar_max(out=nk[:], in0=nk[:], scalar1=EPS)
        invk = small.tile([P, 1], F32)
        v.reciprocal(out=invk[:], in_=nk[:])
        kn = singles.tile([P, Q], F32)
        v.tensor_scalar_mul(out=kn[:], in0=ph[:], scalar1=invk[:])

        psT0 = ps.tile([128, 128], F32, tag="psT")
        te.transpose(out=psT0[:Q, :], in_=kn[:], identity=ident[:])
        knTb = singles.tile([Q, P], BF16)
        sc.copy(out=knTb[:], in_=psT0[:Q, :])
        # block-diagonal KD: KD[32j+q, 128j+a] = knT[q, a] (bf16)
        KD = singles.tile([128, 512], BF16)
        v.memset(KD[:], 0.0)
        for j in range(4):
            nc.sync.dma_start(out=KD[32 * j:32 * (j + 1), 128 * j:128 * (j + 1)],
                              in_=knTb[:])

        # E4[32j+q, j'] = (j == j')  (bf16) - partition-reduce matrix
        E4 = singles.tile([128, 4], BF16)
        v.memset(E4[:], 0.0)
        for j in range(4):
            v.memset(E4[32 * j:32 * (j + 1), j:j + 1], 1.0)

        ones1 = singles.tile([1, 128], F32)
        v.memset(ones1[:], 1.0)

        # PE warm-up: keep the HAM clock hot until real matmuls arrive
        wjunk = singles.tile([128, 128], BF16)
        v.memset(wjunk[:], 0.0)

        def pe_bcast(row_ap, width, tag):
            """Broadcast a [1, width] SBUF row to a [128, width] SBUF tile."""
            pst = psb.tile([128, 512], F32, tag="pst")
            te.matmul(out=pst[:, :width], lhsT=ones1[:, :],
                      rhs=row_ap, start=True, stop=True)
            t = p2.tile([128, width], F32, tag=tag)
            sc.copy(out=t[:], in_=pst[:, :width])
            return t

        def col_to_bcast(col_ap, tag):
            """[128,1] column -> transposed row broadcast to [128,128]."""
            pstx = ps.tile([128, 128], F32, tag="psT")
            te.transpose(out=pstx[:1, :], in_=col_ap, identity=ident[:])
            row = p2.tile([1, 128], F32, tag=tag + "_row")
            sc.copy(out=row[:], in_=pstx[:1, :])
            return pe_bcast(row[:], 128, tag)

        # ------------------------------------------------------------------
        # phase 1: stream qT in; per chunk: norms (E4 matmul), mask matmul,
        # fused scale+relu eviction to bf16, threshold, I-gram accumulation
        # ------------------------------------------------------------------
        psIab = psacc.tile([128, 256], F32, tag="psIab")
        psI_a = psIab[:, 0:128]
        psI_b = psIab[:, 128:256]
        for w in range(16):
            te.matmul(out=psI_a, lhsT=wjunk[:], rhs=wjunk[:],
                      start=True, stop=True, skip_group_check=True)

        mask2 = None
        # groups of 4 chunks: batched norm math (one [128,16] PSUM tile,
        # one sqrt / reciprocal / half pass), then per-chunk mask matmul,
        # PSUM-side threshold, I-gram accumulation and fused evictions
        for t in range(NCHUNK // 4):
            psN4 = psn.tile([128, 16], F32, tag="psN4")
            for g in range(4):
                c = 4 * t + g
                qc = qTall[:, 128 * c:128 * (c + 1)]
                qsq = qpool.tile([128, 128], BF16, tag="qsq")
                gp.tensor_tensor(out=qsq[:], in0=qc, in1=qc, op=AluOp.mult)
                te.matmul(out=psN4[:, 4 * g:4 * g + 4], lhsT=qsq[:], rhs=E4[:],
                          start=True, stop=True)
            sroot4 = small.tile([128, 16], F32, tag="sroot4")
            sc.sqrt(out=sroot4[:], in_=psN4[:])
            inv4 = small.tile([128, 16], F32, tag="inv4")
            v.reciprocal(out=inv4[:], in_=sroot4[:])

            for g in range(4):
                c = 4 * t + g
                qc = qTall[:, 128 * c:128 * (c + 1)]

                # raw mask dots: pm[p, 128j+a]
                pm = psmm.tile([128, 512], F32, tag="pm")
                te.matmul(out=pm[:], lhsT=qc, rhs=KD[:], start=True, stop=True)

                # evict with fused per-partition scale + relu -> bf16
                if c % 2 == 0:
                    mask2 = mpool.tile([128, 1024], BF16, tag="m2")
                mc = mask2[:, 512 * (c % 2):512 * (c % 2) + 512]
                for j in range(4):
                    dst = mc[:, 128 * j:128 * (j + 1)]
                    src = pm[:, 128 * j:128 * (j + 1)]
                    iv = inv4[:, 4 * g + j:4 * g + j + 1]
                    if j < 2:
                        sc.activation(out=dst, in_=src, func=ActFn.Relu,
                                      scale=iv)
                    else:
                        v.tensor_scalar(out=dst, in0=src, scalar1=iv,
                                        scalar2=0.0,
                                        op0=AluOp.mult, op1=AluOp.max)

                if c % 2 == 1:
                    nc.sync.dma_start(out=out_pair_view(c // 2), in_=mask2[:])

                    # binary masks for the chunk pair in one wide op
                    # -> 8 I-gram accumulation matmuls
                    mbc = mbpool.tile([128, 1024], BF16, tag="mb")
                    v.tensor_scalar(out=mbc[:], in0=mask2[:],
                                    scalar1=MASK_THRESH,
                                    scalar2=None, op0=AluOp.is_gt)
                    for j2 in range(8):
                        mbj = mbc[:, 128 * j2:128 * (j2 + 1)]
                        tgt = psI_a if j2 % 2 == 0 else psI_b
                        te.matmul(out=tgt, lhsT=mbj, rhs=mbj,
                                  start=(c == 1 and j2 < 2),
                                  stop=(c == NCHUNK - 1 and j2 >= 6),
                                  skip_group_check=True)

        # ------------------------------------------------------------------
        # phase 1.5: allreduce I within the 4-core batch group
        # ------------------------------------------------------------------
        Ic = singles.tile([128, 128], F32)
        sc.copy(out=Ic[:], in_=psI_a)
        v.tensor_tensor(out=Ic[:], in0=Ic[:], in1=psI_b, op=AluOp.add)
        IS = singles.tile([128, 128], F32)
        if os.environ.get("NMS_NO_COLLECTIVE"):
            v.tensor_copy(out=IS[:], in_=Ic[:])
        else:
            ccin = dram.tile([128, 128], F32)
            ccout = dram.tile([4 * 128, 128], F32)
            nc.sync.dma_start(out=ccin[:], in_=Ic[:])
            gp.collective_compute(
                "AllGather", AluOp.bypass,
                replica_groups=[[0, 1, 2, 3], [4, 5, 6, 7]],
                ins=[ccin[:].opt()], outs=[ccout[:].opt()],
            )
            IS4 = singles.tile([128, 4, 128], F32)
            nc.sync.dma_start(
                out=IS4[:],
                in_=ccout[:].rearrange("(g p) f -> p g f", g=4))
            v.tensor_tensor(out=IS4[:, 0, :], in0=IS4[:, 0, :],
                            in1=IS4[:, 1, :], op=AluOp.add)
            v.tensor_tensor(out=IS4[:, 2, :], in0=IS4[:, 2, :],
                            in1=IS4[:, 3, :], op=AluOp.add)
            v.tensor_tensor(out=IS[:], in0=IS4[:, 0, :],
                            in1=IS4[:, 2, :], op=AluOp.add)

        # ------------------------------------------------------------------
        # compat fitness: bilinear gather of plateau at positions
        # (independent of phase 1; scheduler fills gaps)
        # ------------------------------------------------------------------
        hw = small.tile([P, 2], F32)
        v.tensor_scalar(out=hw[:], in0=posb[:], scalar1=1.0, scalar2=float(H) * 0.5,
                        op0=AluOp.add, op1=AluOp.mult)
        v.tensor_scalar(out=hw[:], in0=hw[:], scalar1=0.0, scalar2=float(H - 1),
                        op0=AluOp.max, op1=AluOp.min)
        rint = small.tile([P, 2], F32)
        v.tensor_scalar(out=rint[:], in0=hw[:], scalar1=TWO23, scalar2=TWO23,
                        op0=AluOp.add, op1=AluOp.subtract)
        gtm = small.tile([P, 2], F32)
        v.tensor_tensor(out=gtm[:], in0=rint[:], in1=hw[:], op=AluOp.is_gt)
        fl = small.tile([P, 2], F32)
        v.tensor_tensor(out=fl[:], in0=rint[:], in1=gtm[:], op=AluOp.subtract)
        cgt = small.tile([P, 2], F32)
        v.tensor_tensor(out=cgt[:], in0=hw[:], in1=fl[:], op=AluOp.is_gt)
        ce = small.tile([P, 2], F32)
        v.tensor_tensor(out=ce[:], in0=fl[:], in1=cgt[:], op=AluOp.add)
        dh = small.tile([P, 2], F32)   # (h-hf, w-wf)
        v.tensor_tensor(out=dh[:], in0=hw[:], in1=fl[:], op=AluOp.subtract)
        dc = small.tile([P, 2], F32)   # (hc-h, wc-w)
        v.tensor_tensor(out=dc[:], in0=ce[:], in1=hw[:], op=AluOp.subtract)

        cw = small.tile([P, 4], F32)   # tl, tr, bl, br weights
        v.tensor_tensor(out=cw[:, 0:1], in0=dc[:, 0:1], in1=dc[:, 1:2], op=AluOp.mult)
        v.tensor_tensor(out=cw[:, 1:2], in0=dc[:, 0:1], in1=dh[:, 1:2], op=AluOp.mult)
        v.tensor_tensor(out=cw[:, 2:3], in0=dh[:, 0:1], in1=dc[:, 1:2], op=AluOp.mult)
        v.tensor_tensor(out=cw[:, 3:4], in0=dh[:, 0:1], in1=dh[:, 1:2], op=AluOp.mult)

        hf256 = small.tile([P, 1], F32)
        v.tensor_scalar_mul(out=hf256[:], in0=fl[:, 0:1], scalar1=float(W))
        hc256 = small.tile([P, 1], F32)
        v.tensor_scalar_mul(out=hc256[:], in0=ce[:, 0:1], scalar1=float(W))
        offf = small.tile([P, 4], F32)  # row index per corner
        v.tensor_tensor(out=offf[:, 0:1], in0=hf256[:], in1=fl[:, 1:2], op=AluOp.add)
        v.tensor_tensor(out=offf[:, 1:2], in0=hf256[:], in1=ce[:, 1:2], op=AluOp.add)
        v.tensor_tensor(out=offf[:, 2:3], in0=hc256[:], in1=fl[:, 1:2], op=AluOp.add)
        v.tensor_tensor(out=offf[:, 3:4], in0=hc256[:], in1=ce[:, 1:2], op=AluOp.add)
        offi = small.tile([P, 4], I32)
        v.tensor_copy(out=offi[:], in_=offf[:])

        G = singles.tile([P, 4, Q], F32)
        for c4 in range(4):
            gp.indirect_dma_start(
                out=G[:, c4, :], out_offset=None,
                in_=plateau,
                in_offset=bass.IndirectOffsetOnAxis(ap=offi[:, c4:c4 + 1], axis=0),
            )

        pv = small.tile([P, Q], F32)
        tmpg = small.tile([P, Q], F32)
        v.tensor_scalar_mul(out=pv[:], in0=G[:, 0, :], scalar1=cw[:, 0:1])
        for c4 in range(1, 4):
            v.tensor_scalar_mul(out=tmpg[:], in0=G[:, c4, :], scalar1=cw[:, c4:c4 + 1])
            v.tensor_tensor(out=pv[:], in0=pv[:], in1=tmpg[:], op=AluOp.add)

        sqp = small.tile([P, Q], F32)
        v.tensor_tensor(out=sqp[:], in0=pv[:], in1=pv[:], op=AluOp.mult)
        npv = small.tile([P, 1], F32)
        v.reduce_sum(out=npv[:], in_=sqp[:], axis=mybir.AxisListType.X)
        sc.sqrt(out=npv[:], in_=npv[:])
        v.tensor_scalar_max(out=npv[:], in0=npv[:], scalar1=EPS)
        invp = small.tile([P, 1], F32)
        v.reciprocal(out=invp[:], in_=npv[:])
        pvn = small.tile([P, Q], F32)
        v.tensor_scalar_mul(out=pvn[:], in0=pv[:], scalar1=invp[:])
        fm = small.tile([P, Q], F32)
        v.tensor_tensor(out=fm[:], in0=kn[:], in1=pvn[:], op=AluOp.mult)
        fit = singles.tile([P, 1], F32)
        v.reduce_sum(out=fit[:], in_=fm[:], axis=mybir.AxisListType.X)

        # winners / losers columns
        wcol = singles.tile([P, 1], F32)
        v.tensor_scalar(out=wcol[:], in0=alive_in[:], scalar1=0.5, scalar2=None,
                        op0=AluOp.is_gt)
        lcol = singles.tile([P, 1], F32)
        v.tensor_scalar(out=lcol[:], in0=wcol[:], scalar1=-1.0, scalar2=1.0,
                        op0=AluOp.mult, op1=AluOp.add)

        fitT_b = col_to_bcast(fit[:], "fitT_b")
        wrow_b = col_to_bcast(wcol[:], "wrow_b")
        lrow_b = col_to_bcast(lcol[:], "lrow_b")

        # ------------------------------------------------------------------
        # pre-collective kill-mask:
        #   kmax_pre[p,q] = ((fit_p < fit_q) & ~(win_p & lose_q)) |
        #                   (lose_p & win_q), zeroed on the diagonal.
        # post-collective, killed[p] = any_q(disputes[p,q] & kmax_pre[p,q])
        # with disputes = 6I > s_p + s_q (I, s exact integers).
        # ------------------------------------------------------------------
        neye = p2.tile([128, 128], F32)
        v.tensor_scalar(out=neye[:], in0=ident[:], scalar1=-1.0, scalar2=1.0,
                        op0=AluOp.mult, op1=AluOp.add)
        t1 = p2.tile([128, 128], F32)
        v.tensor_tensor(out=t1[:], in0=_bcast_free(wcol[:], 128),
                        in1=lrow_b[:], op=AluOp.mult)
        v.tensor_scalar(out=t1[:], in0=t1[:], scalar1=-1.0, scalar2=1.0,
                        op0=AluOp.mult, op1=AluOp.add)
        km = p2.tile([128, 128], F32)
        v.tensor_tensor(out=km[:], in0=_bcast_free(fit[:], 128),
                        in1=fitT_b[:], op=AluOp.is_lt)
        v.tensor_tensor(out=km[:], in0=km[:], in1=t1[:], op=AluOp.mult)
        lw = p2.tile([128, 128], F32)
        v.tensor_tensor(out=lw[:], in0=_bcast_free(lcol[:], 128),
                        in1=wrow_b[:], op=AluOp.mult)
        v.tensor_tensor(out=km[:], in0=km[:], in1=lw[:], op=AluOp.max)
        v.tensor_tensor(out=km[:], in0=km[:], in1=neye[:], op=AluOp.mult)

        # ------------------------------------------------------------------
        # phase 2 (post-collective): disputes -> alive_new
        # ------------------------------------------------------------------
        s_col = p2.tile([128, 1], F32, tag="s_col")
        sdg = p2.tile([128, 128], F32, tag="sdg")
        v.tensor_tensor(out=sdg[:], in0=IS[:], in1=ident[:], op=AluOp.mult)
        v.reduce_sum(out=s_col[:], in_=sdg[:], axis=mybir.AxisListType.X)
        s_row_b = col_to_bcast(s_col[:], "s_row_b")
        ssum = p2.tile([128, 128], F32, tag="ssum")
        v.tensor_tensor(out=ssum[:], in0=_bcast_free(s_col[:], 128),
                        in1=s_row_b[:], op=AluOp.add)
        I6 = p2.tile([128, 128], F32, tag="I6")
        v.tensor_scalar_mul(out=I6[:], in0=IS[:], scalar1=6.0)
        disp = p2.tile([128, 128], F32)
        v.tensor_tensor(out=disp[:], in0=I6[:], in1=ssum[:], op=AluOp.is_gt)
        kfull = p2.tile([128, 128], F32)
        v.tensor_tensor(out=kfull[:], in0=disp[:], in1=km[:], op=AluOp.mult)
        ka = p2.tile([128, 1], F32)
        v.reduce_max(out=ka[:], in_=kfull[:], axis=mybir.AxisListType.X)
        alive_new = p2.tile([128, 1], F32)
        v.tensor_scalar(out=alive_new[:], in0=ka[:], scalar1=-1.0,
                        scalar2=1.0, op0=AluOp.mult, op1=AluOp.add)
        nc.sync.dma_start(out=alive_out, in_=alive_new[:])
        # `out` holds the optimistic (unmasked) masks; the host applies the
        # alive filter with a tiny follow-up kernel only if someone died.

    return nc


def build_apply_alive_kernel():
    """Tiny follow-up kernel: out = masks * alive^T (row-broadcast).
    Only dispatched when the main kernel reports killed agents."""
    nc = bass.Bass("TRN2", target_bir_lowering=False, debug=False,
                   enable_asserts=False, num_devices=N_CORES)
    masks_in = nc.dram_tensor("masks_in", [NCORE_PIX, P], F32,
                              kind="ExternalInput").ap()
    alivev = nc.dram_tensor("alivev", [P, 1], F32, kind="ExternalInput").ap()
    out = nc.dram_tensor("out", [NCORE_PIX, P], F32, kind="ExternalOutput").ap()
    miv = masks_in.rearrange("(c p j) pp -> c p (j pp)", c=NCHUNK, p=128)
    outv = out.rearrange("(c p j) pp -> c p (j pp)", c=NCHUNK, p=128)

    with tile.TileContext(nc) as tc, ExitStack() as ctx:
        singles = ctx.enter_context(tc.tile_pool(name="singles", bufs=1))
        work = ctx.enter_context(tc.tile_pool(name="work", bufs=4))
        psp = ctx.enter_context(tc.tile_pool(name="psp", bufs=2, space="PSUM"))
        v, sc, gp, te = nc.vector, nc.scalar, nc.gpsimd, nc.tensor

        ident = singles.tile([128, 128], F32)
        make_identity(nc, ident[:])
        av = singles.tile([P, 1], F32)
        nc.sync.dma_start(out=av[:], in_=alivev)
        ones1 = singles.tile([1, 128], F32)
        v.memset(ones1[:], 1.0)

        pst = psp.tile([128, 128], F32, tag="pst")
        te.transpose(out=pst[:1, :], in_=av[:], identity=ident[:])
        arow = singles.tile([1, 128], F32)
        sc.copy(out=arow[:], in_=pst[:1, :])
        arow4 = singles.tile([1, 512], F32)
        v.tensor_copy(out=arow4[:],
                      in_=bass.AP(tensor=arow.tensor, offset=arow[:].offset,
                                  ap=[arow[:].ap[0], [0, 4], arow[:].ap[1]]))
        psb = psp.tile([128, 512], F32, tag="psb")
        te.matmul(out=psb[:], lhsT=ones1[:], rhs=arow4[:], start=True, stop=True)
        ab = singles.tile([128, 512], F32)
        sc.copy(out=ab[:], in_=psb[:])

        for c in range(NCHUNK):
            t = work.tile([128, 512], F32, tag="t")
            nc.sync.dma_start(out=t[:], in_=miv[c])
            o = work.tile([128, 512], F32, tag="o")
            v.tensor_tensor(out=o[:], in0=t[:], in1=ab[:], op=AluOp.mult)
            nc.sync.dma_start(out=outv[c], in_=o[:])
    return nc


_NC_CACHE = {}


def _get_nc():
    if "nc" not in _NC_CACHE:
        _NC_CACHE["nc"] = build_kernel()
    return _NC_CACHE["nc"]


def make_in_maps(plateau, phenotypes, positions, alive):
    """Build the 8 per-core input dicts (host-side sharding + layout prep)."""
    pf = plateau.reshape(B, N, Q)
    in_maps = []
    for b in range(B):
        for s in range(NSHARD):
            qs = pf[b, s * NCORE_PIX:(s + 1) * NCORE_PIX]
            # qT[32j+q, 128c+p] = qs[512c + 4p + j, q]
            qT = np.ascontiguousarray(
                qs.reshape(NCHUNK, 128, 4, Q).transpose(2, 3, 0, 1)
                .reshape(128, NCHUNK * 128)).astype(ml_dtypes.bfloat16)
            in_maps.append({
                "qT": qT,
                "plateau": np.ascontiguousarray(pf[b]),
                "phenotypes": np.ascontiguousarray(phenotypes[b]),
                "positions": np.ascontiguousarray(positions[b]),
                "alive": np.ascontiguousarray(alive[b]),
            })
    return in_maps


def kernel(plateau, phenotypes, positions, alive):
    nc = _get_nc()
    plateau = np.ascontiguousarray(plateau, dtype=np.float32)
    phenotypes = np.ascontiguousarray(phenotypes, dtype=np.float32)
    positions = np.ascontiguousarray(positions, dtype=np.float32)
    alive = np.ascontiguousarray(alive, dtype=np.float32)

    in_maps = make_in_maps(plateau, phenotypes, positions, alive)
    res = bass_utils.run_bass_kernel_spmd(
        nc, in_maps, core_ids=list(range(N_CORES)))
    out = np.empty((B, N, P), dtype=np.float32)
    for b in range(B):
        for s in range(NSHARD):
            out[b, s * NCORE_PIX:(s + 1) * NCORE_PIX] = \
                res.results[b * NSHARD + s]["out"].astype(np.float32)

    # apply the alive filter on-device if any agent was killed (rare)
    alive_new = [res.results[b * NSHARD]["alive_out"] for b in range(B)]
    if any((a < 0.5).any() for a in alive_new):
        if "nc2" not in _NC_CACHE:
            _NC_CACHE["nc2"] = build_apply_alive_kernel()
        nc2 = _NC_CACHE["nc2"]
        in_maps2 = []
        for b in range(B):
            for s in range(NSHARD):
                in_maps2.append({
                    "masks_in": np.ascontiguousarray(
                        out[b, s * NCORE_PIX:(s + 1) * NCORE_PIX]),
                    "alivev": alive_new[b],
                })
        res2 = bass_utils.run_bass_kernel_spmd(
            nc2, in_maps2, core_ids=list(range(N_CORES)))
        for b in range(B):
            for s in range(NSHARD):
                out[b, s * NCORE_PIX:(s + 1) * NCORE_PIX] = \
                    res2.results[b * NSHARD + s]["out"]
    return out

